# revision 62
# baseline (speedup 1.0000x reference)
"""Trainium2 Bass kernel for nn_AttentionMLP (B=4, S=4096, two attention+MLP
stages).

Sharding: 8 cores = 4 batches x 2 sequence-halves. Each core computes its
2048 query rows end-to-end. Stage-1 output halves are exchanged pairwise so
stage 2 attends over the full sequence.

Key structure (v3, software-pipelined, ACT-bound by design):
  - Per 512-query chunk: scoresT blocks (PE) -> exp (ACT) -> attn@v+rowsum
    (PE, ones-augmented lhsT) accumulate in PSUM. The attn@v of group g is
    emitted AFTER the scores of group g+1 (one-group lag), so PE never
    blocks the score feed while waiting for an exp; the last group's attn@v
    plus the PSUM drain spill into the next chunk's slot 0.
  - The normalize+MLP tail of chunk n is split into small parts injected
    between chunk n+1's score groups at hand-tuned points, so the ACT
    engine (the bottleneck: all the softmax exps) never waits on the tail.
  - The output projection accumulates W2*relu-part and W2*exp-part
    separately in PSUM (no hT = r+e intermediate), with the bias applied by
    a per-partition tensor_scalar during the PSUM drain.
  - Normalize: reciprocal_approx_fast + K=1 ones-matmul broadcast on PE.
  - Exchange: outT chunk -> DRAM bounce -> pairwise AllReduce(add) ->
    pull + DVE subtract of own half = peer half (SPMD-symmetric, no
    core-dependent addressing). Stage-2's own-half K/V are projected
    straight from outT in SBUF during stage 1; peer-half key blocks are
    consumed late in stage-2's first chunk to hide the exchange tail.
  - Output is written feature-major [64, R]; the host transposes.
  - All DMAs ride the SP/Pool queues; ACT/PE/DVE queues carry only compute.
"""

import os
import numpy as np
from contextlib import ExitStack

import concourse.bass as bass
import concourse.tile as tile
from concourse import bacc, mybir
from concourse import bass_utils

F32 = mybir.dt.float32
F32R = mybir.dt.float32r
EXP = mybir.ActivationFunctionType.Exp
ADD = mybir.AluOpType.add
MIN = mybir.AluOpType.min
MAX = mybir.AluOpType.max
MULT = mybir.AluOpType.mult
I32 = mybir.dt.int32
# Schraudolph fast-exp: exp(x) ~= bitcast_f32(int32(SCH_A*x + SCH_B));
# one chunk-group's exp per chunk rides the (otherwise idle) DVE this way,
# cutting the ACT-engine floor. Softmax normalization cancels the
# common-mode part of the ~1.5% weight noise.
SCH_A = 12102203.1616
SCH_B = 1064866805.0
DVE_EXP_GI = 8          # which group of a standard chunk goes to DVE

N_CORES = 8
B, S, D = 4, 4096, 64
R = S // 2            # own query rows per core
HD = 256
NCK = R // 512        # si-chunks per core (4 x 512)
NJB = S // 128        # key blocks (32 x 128)
GROUPS = [3] * 10 + [2]            # key-block group sizes per chunk
GROUPS_S1C0 = [1, 2] + [3] * 9 + [2]   # ramp: first exp fires ASAP
GROUPS_S2C0 = [3] * 8 + [2, 2, 2, 2]  # peer-dependent blocks last
RG = [[0, 1], [2, 3], [4, 5], [6, 7]]

# packed-weight column layout (f32 words per partition)
# region A (partitions 0-63, one 448-col block per stage): wq|wk|wv|w1t
WQ0, WK0, WV0, W1T0 = 0, 64, 128, 192
RA = 896
# region B: w2t (2 stages x 2 K-blocks x 64) | b1 cols (2x2) | b2 cols (2)
W2T0, B1C0, B2C0 = RA, RA + 256, RA + 260
WCOLS = RA + 262

# tail-part injection points (group index in the NEXT chunk):
#   t1 rb/aT, t2 hidden, t3 ELU-exp, t4r W2*r, t4e W2*e+out, q/k/v emits
# t3 (the MLP exp) fills the ACT hole left by the DVE-offloaded group @8
TAIL_PTS = [3, 4, 8, 6, 9, 9]       # t1 t2 t3 t4r t4e t4qk
TAIL_PTS_S2 = [3, 4, 8, 6, 9]
# the stage-1 last-chunk tail injects early into stage-2 chunk 0 so the
# exchange of the last outT chunk fires with slack before its consumers
BOUNDARY_PTS = [1, 2, 3, 4, 5, 5]


def build_nc(n_cores=N_CORES, reps=1):
    nc = bacc.Bacc("TRN2", target_bir_lowering=False, debug=False,
                   num_devices=n_cores)

    xT_d = nc.dram_tensor("xT", [64, S], F32R, kind="ExternalInput").ap()
    w_d = nc.dram_tensor("wpack", [128, WCOLS], F32R,
                         kind="ExternalInput").ap()
    out_d = nc.dram_tensor("out1", [64, R], F32, kind="ExternalOutput").ap()

    with tile.TileContext(nc) as tc, ExitStack() as ctx:
        consts = ctx.enter_context(tc.tile_pool(name="consts", bufs=1))
        sb = ctx.enter_context(tc.tile_pool(name="sb", bufs=1))
        ps = ctx.enter_context(tc.tile_pool(name="ps", bufs=2, space="PSUM"))
        dram = ctx.enter_context(tc.tile_pool(name="dram", bufs=1,
                                              space="DRAM"))

        wt = consts.tile([128, WCOLS], F32R)
        nc.sync.dma_start(wt[:, 0:224], w_d[:, 0:224])
        nc.sync.dma_start(wt[:, 224:448], w_d[:, 224:448])
        ones_f32 = consts.tile([1, 64], F32)
        nc.vector.memset(ones_f32[:], 1.0)
        ones64 = consts.tile([1, 64], F32R)
        nc.vector.tensor_copy(ones64[:], ones_f32[:])
        # dummy exp: forces the ACT table load during the input-DMA window
        # instead of on the first real exp's critical path
        warm = consts.tile([1, 8], F32)
        nc.scalar.activation(warm[:], ones_f32[:, 0:8], EXP)

        wsl = [wt[0:64, sfx * 448:sfx * 448 + 448] for sfx in (0, 1)]
        w2t = [wt[:, W2T0 + sfx * 128:W2T0 + (sfx + 1) * 128]
               for sfx in (0, 1)]
        b1c = [wt[:, B1C0 + sfx * 2:B1C0 + sfx * 2 + 2].bitcast(F32)
               for sfx in (0, 1)]
        b2c = [wt[0:64, B2C0 + sfx:B2C0 + sfx + 1].bitcast(F32)
               for sfx in (0, 1)]

        for _rep in range(reps):
            rn = f"r{_rep}"
            xT = sb.tile([64, S], F32R, tag="xt", bufs=2, name=f"xT_{rn}")
            # slice 0 on the gpsimd queue so the first projection's two
            # inputs (wt region A on sync, xT slice 0) arrive in parallel
            for n in range(S // 512):
                eng = nc.gpsimd if n % 2 == 0 else nc.sync
                eng.dma_start(xT[:, n * 512:(n + 1) * 512],
                              xT_d[:, n * 512:(n + 1) * 512])
            if _rep == 0:
                nc.gpsimd.dma_start(wt[:, 448:WCOLS], w_d[:, 448:WCOLS])

            outT = sb.tile([64, R], F32R, tag="outT", name=f"oT_{rn}")
            xs_tiles = {}
            xT2o = sb.tile([64, R], F32R, tag="x2o", name=f"x2_{rn}")
            bi = [dram.tile([64, 512], F32R, tag=f"bi{n}",
                            name=f"bi_{rn}_{n}") for n in range(NCK)]
            br = [dram.tile([2, 64, 512], F32R, tag=f"br{n}",
                            name=f"br_{rn}_{n}") for n in range(NCK)]

            def mkproj(sfx):
                qT = sb.tile([64, R], F32R, tag="qT", bufs=2,
                             name=f"qT{sfx}_{rn}")
                kT = sb.tile([64, S], F32R, tag="kT", bufs=2,
                             name=f"kT{sfx}_{rn}")
                va = sb.tile([128, NJB, 65], F32R, tag="va", bufs=2,
                             name=f"va{sfx}_{rn}")
                oc = sb.tile([128, NJB], F32, tag="oc", bufs=2,
                             name=f"oc{sfx}_{rn}")
                nc.vector.memset(oc[:], 1.0)
                nc.vector.tensor_copy(va[:, :, 64:65], oc[:].unsqueeze(2))
                return qT, kT, va

            proj = [mkproj(0), mkproj(1)]

            sched = {}

            def add_sched(st_i, ch, g, fn):
                sched.setdefault((st_i, ch, g), []).append(fn)

            def emit_k(sfx, ksl, src, dst=None, ptag="sA", pbufs=2):
                # project keys for 512-key slice dst from src [64,512]
                d = ksl if dst is None else dst
                kT_ = proj[sfx][1]
                pk = ps.tile([64, 512], F32, tag=ptag, bufs=pbufs)
                nc.tensor.matmul(pk[:], wsl[sfx][:, WK0:WK0 + 64], src,
                                 start=True, stop=True)
                nc.vector.tensor_copy(kT_[:, d * 512:(d + 1) * 512], pk[:])

            def emit_v(sfx, ksl, src, dst=None, ptag="sA", pbufs=2):
                d = ksl if dst is None else dst
                va_ = proj[sfx][2]
                pv = ps.tile([128, 4, 64], F32, tag=ptag, bufs=pbufs)
                for i in range(4):
                    nc.tensor.matmul(pv[:, i, :],
                                     src[:, i * 128:(i + 1) * 128],
                                     wsl[sfx][:, WV0:WV0 + 64],
                                     start=True, stop=True)
                nc.vector.tensor_copy(va_[:, d * 4:(d + 1) * 4, 0:64], pv[:])

            def emit_kv(sfx, ksl, src, dst=None):
                emit_k(sfx, ksl, src, dst)
                emit_v(sfx, ksl, src, dst)

            def emit_q(sfx, n2, src, ptag="sA", pbufs=2):
                qT_ = proj[sfx][0]
                pq = ps.tile([64, 512], F32, tag=ptag, bufs=pbufs)
                nc.tensor.matmul(pq[:], wsl[sfx][:, WQ0:WQ0 + 64], src,
                                 start=True, stop=True)
                nc.vector.tensor_copy(qT_[:, n2 * 512:(n2 + 1) * 512], pq[:])

            def emit_qk(sfx, n2, ksl, src, first=False,
                        ptag="sA", pbufs=2):
                # Wq|Wk are adjacent in region A: one [64,128] lhsT yields
                # the q-projection (rows 0-63) AND k-projection (64-127)
                # of the same source slice in a single matmul
                qT_, kT_, _ = proj[sfx]
                pqk = ps.tile([128, 512], F32, tag=ptag, bufs=pbufs)
                nc.tensor.matmul(pqk[:], wsl[sfx][:, WQ0:WQ0 + 128], src,
                                 start=True, stop=True)
                nc.vector.tensor_copy(qT_[:, n2 * 512:(n2 + 1) * 512],
                                      pqk[0:64, :])
                if first:   # startup: ACT is idle, parallelize the copies
                    nc.scalar.copy(kT_[:, ksl * 512:(ksl + 1) * 512],
                                   pqk[64:128, :])
                else:
                    nc.vector.tensor_copy(kT_[:, ksl * 512:(ksl + 1) * 512],
                                          pqk[64:128, :])

            def make_tail(sfx, n2, dr, c0=0, cw=512, ptag="mlp", pbufs=1):
                """Normalize + MLP + output for columns [c0, c0+cw) of
                chunk n2. dr supplies the drained rs/rr/araw tiles at
                emission time. Parts are injected between the next chunk's
                score groups."""
                stt = {}
                osl = slice(n2 * 512 + c0, n2 * 512 + c0 + cw)
                csl = slice(c0, c0 + cw)

                def t1():  # broadcast 1/rowsum across partitions, normalize
                    rb = ps.tile([64, cw], F32, tag=ptag, bufs=pbufs)
                    nc.tensor.matmul(rb[:], ones64[:], dr['rr'][:, csl],
                                     start=True, stop=True)
                    aT = sb.tile([64, cw], F32R, tag="aT", bufs=2)
                    nc.vector.scalar_tensor_tensor(
                        aT[:], rb[:], 0.0, dr['araw'][:, csl],
                        op0=ADD, op1=mybir.AluOpType.mult)
                    stt['aT'] = aT

                def t2():  # hidden pre-activations, ELU pieces on DVE
                    u = sb.tile([128, 2 * cw], F32, tag="u", bufs=2)
                    r = sb.tile([128, 2 * cw], F32R, tag="r2", bufs=2)
                    for j in range(2):
                        ph = ps.tile([128, cw], F32, tag=ptag, bufs=pbufs)
                        nc.tensor.matmul(
                            ph[:],
                            wsl[sfx][:, W1T0 + j * 128:W1T0 + (j + 1) * 128],
                            stt['aT'][:], start=True, stop=True)
                        nc.vector.tensor_scalar(
                            u[:, j * cw:(j + 1) * cw], ph[:],
                            b1c[sfx][:, j:j + 1], 0.0, op0=ADD, op1=MIN)
                        nc.vector.tensor_scalar(
                            r[:, j * cw:(j + 1) * cw], ph[:],
                            b1c[sfx][:, j:j + 1], 0.0, op0=ADD, op1=MAX)
                    stt['u'], stt['r'] = u, r

                def t3():  # the ELU exp (ACT)
                    e = sb.tile([128, 2 * cw], F32R, tag="e", bufs=2)
                    nc.scalar.activation(e[:], stt['u'][:], EXP)
                    stt['e'] = e

                def t4r():  # output projection, relu part
                    po = ps.tile([64, cw], F32, tag=ptag, bufs=pbufs)
                    for j in range(2):
                        nc.tensor.matmul(po[:],
                                         w2t[sfx][:, j * 64:(j + 1) * 64],
                                         stt['r'][:, j * cw:(j + 1) * cw],
                                         start=(j == 0), stop=False)
                    stt['po'] = po

                def t4e():  # output projection exp part, bias, ship
                    po = stt['po']
                    for j in range(2):
                        nc.tensor.matmul(po[:],
                                         w2t[sfx][:, j * 64:(j + 1) * 64],
                                         stt['e'][:, j * cw:(j + 1) * cw],
                                         start=False, stop=(j == 1))
                    if sfx == 0:
                        nc.vector.tensor_scalar(outT[:, osl], po[:],
                                                b2c[0], 0.0,
                                                op0=ADD, op1=ADD)
                        # exchange this chunk: AllGather both halves;
                        # peer = (m0 + m1) - own, computed at the use site
                        nc.sync.dma_start(bi[n2][:], outT[:, osl])
                        if n_cores > 1 and not os.environ.get("BASS_NO_CC"):
                            nc.gpsimd.collective_compute(
                                "AllGather", mybir.AluOpType.bypass,
                                replica_groups=RG,
                                ins=[bi[n2][:].opt()],
                                outs=[br[n2][:].opt()])
                        else:
                            for m in range(2):
                                nc.sync.dma_start(br[n2][m], bi[n2][:])
                        xs = sb.tile([64, 2, cw], F32R, tag="xs", bufs=2,
                                     name=f"xs_{rn}_{n2}")
                        nc.sync.dma_start(xs[:, 0, :], br[n2][0])
                        nc.sync.dma_start(xs[:, 1, :], br[n2][1])
                        xs_tiles[n2] = xs
                    else:
                        fin = sb.tile([64, cw], F32, tag="fin", bufs=2)
                        nc.vector.tensor_scalar(fin[:], po[:], b2c[1], 0.0,
                                                op0=ADD, op1=ADD)
                        nc.sync.dma_start(out_d[:, osl], fin[:])

                # stage-2 projections of this chunk (own half), one PE
                # detour per injection point
                def t4qk():
                    emit_qk(1, n2, n2, outT[:, osl], ptag="mlp", pbufs=1)

                def t4v():
                    emit_v(1, n2, outT[:, osl], ptag="mlp", pbufs=1)

                if sfx == 0:
                    return [t1, t2, t3, t4r, t4e, t4qk, t4v]
                return [t1, t2, t3, t4r, t4e]

            def sub_fn(k):
                # peer half = (m0 + m1) minus own contribution
                sl = slice(k * 512, (k + 1) * 512)

                def emit():
                    xs = xs_tiles[k]
                    xm = sb.tile([64, 512], F32R, tag="xm", bufs=2,
                                 name=f"xm_{rn}_{k}")
                    nc.vector.tensor_add(xm[:], xs[:, 0, :], xs[:, 1, :])
                    nc.vector.tensor_sub(xT2o[:, sl], xm[:], outT[:, sl])
                return emit

            def run_stage(sfx):
                qT_, kT_, va_ = proj[sfx]
                for n2 in range(NCK):
                    qs = qT_[:, n2 * 512:(n2 + 1) * 512]
                    if sfx == 1 and n2 == 0:
                        order = (list(range(12)) + list(range(16, 28)) +
                                 list(range(12, 16)) + list(range(28, 32)))
                        groups = GROUPS_S2C0
                    elif sfx == 0 and n2 == 0:
                        order = list(range(NJB))
                        groups = GROUPS_S1C0
                    else:
                        order = list(range(NJB))
                        groups = GROUPS
                    avh = [None]    # chunk's [65,512] PSUM accumulator
                    pend = []       # (jbs, ex, idx0, due_gi) awaiting attn@v

                    def flush_av(gi_now, avh=avh, pend=pend):
                        while pend and pend[0][3] <= gi_now:
                            jbs_, ex_, i0, _ = pend.pop(0)
                            for i, jb in enumerate(jbs_):
                                nc.tensor.matmul(
                                    avh[0][:], va_[:, jb, :],
                                    ex_[:, i * 512:(i + 1) * 512],
                                    start=(i0 + i == 0),
                                    stop=(i0 + i == NJB - 1))

                    idx = 0
                    for gi, gsz in enumerate(groups):
                        jbs = order[idx:idx + gsz]
                        # at a chunk seam, emit the first score group BEFORE
                        # the slot-0 housekeeping (leftover attn@v + drain),
                        # so ACT's first exp isn't starved behind PE's
                        # catch-up work. (Chunk (0,0) needs its emits first.)
                        early_st = gi == 0 and (sfx, n2) != (0, 0)
                        if early_st:
                            st = ps.tile([128, gsz * 512], F32, tag="sA",
                                         bufs=2, name=f"st_{sfx}_{n2}")
                            for i, jb in enumerate(jbs):
                                nc.tensor.matmul(
                                    st[:, i * 512:(i + 1) * 512],
                                    kT_[:, jb * 128:(jb + 1) * 128], qs,
                                    start=True, stop=True)
                        for fn in sched.pop((sfx, n2, gi), ()):
                            fn()
                        if avh[0] is None:
                            # allocated after the slot-0 fns so the WAR on
                            # the previous chunk's drain is recorded
                            avh[0] = ps.tile([65, 512], F32, tag="av",
                                             bufs=1, name=f"av{sfx}_{n2}")
                        if not early_st:
                            st = ps.tile([128, gsz * 512], F32, tag="sA",
                                         bufs=2)
                            for i, jb in enumerate(jbs):
                                nc.tensor.matmul(
                                    st[:, i * 512:(i + 1) * 512],
                                    kT_[:, jb * 128:(jb + 1) * 128], qs,
                                    start=True, stop=True)
                        on_dve = (groups is GROUPS and gi == DVE_EXP_GI)
                        if on_dve:
                            # Schraudolph exp on DVE; extra group of attn@v
                            # lag hides the slower chain
                            w = gsz * 512
                            exi = sb.tile([128, w], I32, tag="exi", bufs=1)
                            nc.vector.tensor_scalar(exi[:], st[:],
                                                    SCH_A, SCH_B,
                                                    op0=MULT, op1=ADD)
                            ex = sb.tile([128, w], F32R, tag="exp", bufs=3)
                            nc.vector.tensor_copy(ex[:], exi[:].bitcast(F32))
                        else:
                            ex = sb.tile([128, gsz * 512], F32R, tag="exp",
                                         bufs=3)
                            nc.scalar.activation(ex[:], st[:], EXP)
                        flush_av(gi - 1)   # attn@v with one-group lag
                        pend.append((jbs, ex, idx, gi + (2 if on_dve else 1)))
                        idx += gsz

                    # leftover: last group's attn@v + PSUM drain + recip,
                    # deferred into the next chunk's slot 0
                    dr = {}

                    def leftover(avh=avh, flush_av=flush_av, dr=dr):
                        flush_av(99)
                        av = avh[0]
                        rs = sb.tile([1, 512], F32, tag="rs", bufs=2)
                        nc.vector.tensor_copy(rs[:], av[64:65, :])
                        rr = sb.tile([1, 512], F32, tag="rr", bufs=2)
                        nc.vector.reciprocal_approx_fast(rr[:], rs[:])
                        rrr = sb.tile([1, 512], F32R, tag="rrr", bufs=2)
                        nc.vector.tensor_copy(rrr[:], rr[:])
                        araw = sb.tile([64, 512], F32, tag="araw", bufs=2)
                        nc.vector.tensor_copy(araw[:], av[0:64, :])
                        dr['rr'], dr['araw'] = rrr, araw

                    if sfx == 0 and n2 == NCK - 1:
                        add_sched(1, 0, 0, leftover)
                        parts = make_tail(sfx, n2, dr)
                        for g, p in zip(BOUNDARY_PTS, parts[:6]):
                            add_sched(1, 0, g, p)
                        add_sched(1, 0, 7, parts[6])
                    elif sfx == 1 and n2 == NCK - 1:
                        # final tail: two pipelined column halves to shrink
                        # the serial end-of-kernel chain
                        leftover()
                        pa = make_tail(sfx, n2, dr, 0, 256,
                                       ptag="sA", pbufs=2)
                        pb = make_tail(sfx, n2, dr, 256, 256,
                                       ptag="sA", pbufs=2)
                        pa[0](), pb[0](), pa[1](), pa[2]()
                        pb[1](), pa[3](), pa[4](), pb[2]()
                        pb[3](), pb[4]()
                    else:
                        add_sched(sfx, n2 + 1, 0, leftover)
                        parts = make_tail(sfx, n2, dr)
                        if sfx == 0:
                            for g, p in zip(TAIL_PTS, parts[:6]):
                                add_sched(0, n2 + 1, g, p)
                            add_sched(0, n2 + 1, 10, parts[6])
                        else:
                            for g, p in zip(TAIL_PTS_S2, parts):
                                add_sched(1, n2 + 1, g, p)

            # stage-1 emits in chunk 0: slices 0-3 use the merged q+k
            # projection (all four q chunk-slices done here)
            for g in range(8):
                def c0_emit(g=g):
                    src_ = xT[:, g * 512:(g + 1) * 512]
                    if g < 4:
                        emit_qk(0, g, g, src_, first=(g == 0))
                    else:
                        emit_k(0, g, src_)
                    emit_v(0, g, src_, ptag="mlp", pbufs=1)
                add_sched(0, 0, g, c0_emit)
            # peer-half subtract scheduled where its pull is long done
            add_sched(0, 3, 0, sub_fn(0))

            run_stage(0)

            # stage-2 peer-half K/V emits (consume the exchanged halves)
            add_sched(1, 0, 0, sub_fn(1))
            add_sched(1, 0, 3, sub_fn(2))
            add_sched(1, 0, 7, sub_fn(3))
            for k, g in zip(range(4), [1, 2, 4, 9]):
                add_sched(1, 0, g,
                          (lambda k=k: emit_kv(
                              1, None, xT2o[:, k * 512:(k + 1) * 512],
                              dst=4 + k)))

            run_stage(1)
            assert not sched, f"unconsumed sched entries: {list(sched)}"

    nc.compile()
    return nc


def prep_inputs(x, q, k, v, q1, k1, v1, W1, b1, W2, b2, W11, b11, W22, b22):
    """Returns per-core in_maps for run_bass_kernel_spmd."""
    f = np.float32

    def cast(a):
        return np.ascontiguousarray(np.asarray(a), dtype=f)

    scale = f(1.0 / np.sqrt(np.float32(64)))
    wpack = np.zeros((128, WCOLS), dtype=f)
    for sfx, (qq, kk, vv, W1_, b1_, W2_, b2_) in enumerate(
            [(q, k, v, W1, b1, W2, b2), (q1, k1, v1, W11, b11, W22, b22)]):
        c0 = 448 * sfx
        wpack[0:64, c0 + WQ0:c0 + WQ0 + 64] = cast(qq) * scale
        wpack[0:64, c0 + WK0:c0 + WK0 + 64] = cast(kk)
        wpack[0:64, c0 + WV0:c0 + WV0 + 64] = cast(vv)
        wpack[0:64, c0 + W1T0:c0 + W1T0 + HD] = cast(W1_).T
        w2T = cast(W2_).T                                 # [HD, 64]
        for j in range(2):
            wpack[:, W2T0 + sfx * 128 + j * 64:
                  W2T0 + sfx * 128 + (j + 1) * 64] = w2T[j * 128:(j + 1) * 128]
            wpack[:, B1C0 + sfx * 2 + j] = cast(b1_)[j * 128:(j + 1) * 128]
        # ELU is computed as (elu+1); fold the -1 into an effective b2
        wpack[0:64, B2C0 + sfx] = cast(b2_) - cast(W2_).sum(axis=1)

    in_maps = []
    xc = cast(x)
    for c in range(N_CORES):
        b, h = c // 2, c % 2
        xb = xc[b]                      # [S, 64]
        if h == 1:                      # own half first
            xb = np.concatenate([xb[R:], xb[:R]], axis=0)
        in_maps.append({"xT": np.ascontiguousarray(xb.T), "wpack": wpack})
    return in_maps


_NC_CACHE = None


def kernel(**inputs) -> np.ndarray:
    global _NC_CACHE
    if _NC_CACHE is None:
        _NC_CACHE = build_nc()
    nc = _NC_CACHE
    in_maps = prep_inputs(**inputs)
    res = bass_utils.run_bass_kernel_spmd(nc, in_maps,
                                          core_ids=list(range(N_CORES)))
    out = np.empty((B, S, 64), dtype=np.float32)
    for c in range(N_CORES):
        b, h = c // 2, c % 2
        out[b, h * R:(h + 1) * R, :] = res.results[c]["out1"].T
    return out
